# revision 9
# baseline (speedup 1.0000x reference)
"""DCRNN encoder (2-layer DCGRU, K=2 chebyshev diffusion) on 8 TRN2 NeuronCores.

Data-parallel over batch (B=64 -> 8 per core).  Math:
  gconv(cat(x,h)) = sum_m P_m @ cat @ W_m  with P_0=I, P_1=S, P_2=2S^2-I.
  The -I of P_2 is folded into W'_0 = W_0 - W_2 host-side, so the diffusion
  terms are [z, S z, 2S^2 z].  W rows are split into x-part and h-part: the
  x-part of layer 0 is precomputed for all timesteps in a bulk phase (XG0/XC0
  via DRAM); layer 1's x-part is computed in-round from layer 0's fresh state
  (SBUF only).  On-chip tensors are feature-major (feat, b, n); chebyshev
  stationaries are node-major, produced by PE transpose matmuls.  bf16 on PE,
  fp32 PSUM.

Round pipeline (keeps the PE dense so HAM stays at full clock):
  round t:  hnm0 = transpose(h0)            # h0 = y0_{t-1} after round t-1
            L1bulk(t-1)  from (h0, hnm0)    # -> xg1/xc1 SBUF tiles
            L0 step t    (XG0[t] from DRAM) # updates h0
            L1 step t-1  (xg1/xc1)          # updates h1, emits outputs[t-2]
"""
import numpy as np
import ml_dtypes

import concourse.bass as bass
import concourse.tile as tile
from concourse import bacc, mybir
from concourse.bass_utils import run_bass_kernel_spmd

BF16 = mybir.dt.bfloat16
F32 = mybir.dt.float32

T, BFULL, N, D, H = 128, 64, 128, 64, 128
NCORES = 8
B = BFULL // NCORES          # 8 per core
TOK = B * N                  # 1024 tokens per core
nbf = np.dtype(ml_dtypes.bfloat16)
SIG = mybir.ActivationFunctionType.Sigmoid
TANH = mybir.ActivationFunctionType.Tanh


def _prep_weights(supports, Wg0, Wc0, Wg1, Wc1):
    S = np.asarray(supports)[0].astype(np.float32)           # (N, N)
    S2 = 2.0 * (S @ S)
    RC = np.concatenate([S.T, S2.T], axis=1)                 # (N, 2N)
    out = {"RC": RC.astype(nbf), "IDENT": np.eye(N, dtype=np.float32).astype(nbf)}

    def split(W, Dl):
        W = np.asarray(W).astype(np.float32)
        Wm = [W[m::3] for m in range(3)]                      # (F, out)
        W0p = Wm[0] - Wm[2]
        xs = np.concatenate([W0p[:Dl], Wm[1][:Dl], Wm[2][:Dl]], axis=0)
        hs = np.concatenate([W0p[Dl:], Wm[1][Dl:], Wm[2][Dl:]], axis=0)
        return xs.astype(nbf), hs.astype(nbf)

    out["Wg0x"], out["Wg0h"] = split(Wg0, D)
    out["Wc0x"], out["Wc0h"] = split(Wc0, D)
    out["Wg1x"], out["Wg1h"] = split(Wg1, H)
    out["Wc1x"], out["Wc1h"] = split(Wc1, H)
    return out


def _wchunks(rows, size):
    chunks, r0 = [], 0
    while r0 < rows:
        r = min(size, rows - r0)
        chunks.append((r0, r))
        r0 += r
    return chunks


class _Builder:
    def __init__(self):
        nc = bacc.Bacc("TRN2", target_bir_lowering=False, debug=False,
                       num_devices=NCORES)
        self.nc = nc
        dram = lambda name, shape, dt, **kw: nc.dram_tensor(name, shape, dt, **kw).ap()

        self.x = dram("x", [T, B, N, D], BF16, kind="ExternalInput")
        self.xT = dram("xT", [T, D, B, N], BF16, kind="ExternalInput")
        self.h0fm = dram("h0fm", [2, H, B, N], BF16, kind="ExternalInput")
        self.RC = dram("RC", [N, 2 * N], BF16, kind="ExternalInput")
        self.IDENT = dram("IDENT", [N, N], BF16, kind="ExternalInput")
        for nm, shape in [("Wg0x", [3 * D, 2 * H]), ("Wc0x", [3 * D, H]),
                          ("Wg0h", [3 * H, 2 * H]), ("Wc0h", [3 * H, H]),
                          ("Wg1x", [3 * H, 2 * H]), ("Wc1x", [3 * H, H]),
                          ("Wg1h", [3 * H, 2 * H]), ("Wc1h", [3 * H, H])]:
            setattr(self, nm, dram(nm, shape, BF16, kind="ExternalInput"))
        self.bg0 = dram("bg0", [2 * H], F32, kind="ExternalInput")
        self.bc0 = dram("bc0", [H], F32, kind="ExternalInput")
        self.bg1 = dram("bg1", [2 * H], F32, kind="ExternalInput")
        self.bc1 = dram("bc1", [H], F32, kind="ExternalInput")

        self.out_hidden = dram("out_hidden", [2, B, N * H], F32, kind="ExternalOutput")
        self.outputs = dram("outputs", [T, B, N * H], F32, kind="ExternalOutput")

    def build(self):
        nc = self.nc
        with tile.TileContext(nc) as tc:
            self._emit(tc)
        nc.compile()
        return nc

    # ---------------- consts ----------------

    def _load_const_tiles(self, pool):
        nc = self.nc
        g = {}
        g["RC"] = pool.tile([N, 2 * N], BF16, tag="cRC", name="cRC")
        nc.sync.dma_start(out=g["RC"], in_=self.RC)
        g["I"] = pool.tile([N, N], BF16, tag="cI", name="cI")
        nc.sync.dma_start(out=g["I"], in_=self.IDENT)

        def wtiles(ap, rows, cols, key, csize):
            tiles = []
            for (r0, r) in _wchunks(rows, csize):
                row = []
                for m0 in range(0, cols, 128):
                    t = pool.tile([r, 128], BF16, tag=f"w{key}_{r0}_{m0}",
                                  name=f"w{key}_{r0}_{m0}")
                    nc.sync.dma_start(out=t, in_=ap[r0:r0 + r, m0:m0 + 128])
                    row.append(t)
                tiles.append(row)
            return tiles

        g["Wg0x"] = wtiles(self.Wg0x, 3 * D, 2 * H, "g0x", D)
        g["Wc0x"] = wtiles(self.Wc0x, 3 * D, H, "c0x", D)
        g["Wg0h"] = wtiles(self.Wg0h, 3 * H, 2 * H, "g0h", H)
        g["Wc0h"] = wtiles(self.Wc0h, 3 * H, H, "c0h", H)
        g["Wg1x"] = wtiles(self.Wg1x, 3 * H, 2 * H, "g1x", H)
        g["Wc1x"] = wtiles(self.Wc1x, 3 * H, H, "c1x", H)
        g["Wg1h"] = wtiles(self.Wg1h, 3 * H, 2 * H, "g1h", H)
        g["Wc1h"] = wtiles(self.Wc1h, 3 * H, H, "c1h", H)

        bgt0 = pool.tile([128, 2], F32, tag="bg0", name="bg0")
        nc.sync.dma_start(out=bgt0, in_=self.bg0.rearrange("(k p) -> p k", p=128))
        bct0 = pool.tile([128, 1], F32, tag="bc0", name="bc0")
        nc.sync.dma_start(out=bct0, in_=self.bc0.rearrange("(k p) -> p k", p=128))
        bgt1 = pool.tile([128, 2], F32, tag="bg1", name="bg1")
        nc.sync.dma_start(out=bgt1, in_=self.bg1.rearrange("(k p) -> p k", p=128))
        bct1 = pool.tile([128, 1], F32, tag="bc1", name="bc1")
        nc.sync.dma_start(out=bct1, in_=self.bc1.rearrange("(k p) -> p k", p=128))
        g["bg"] = [bgt0, bgt1]
        g["bc"] = [bct0, bct1]
        return g

    # ---------------- pieces ----------------

    def _w_contract(self, psum, wtiles, rhs_chunks, extra=None):
        """psum[mt][:, half] += sum_c wtiles[c][mt].T @ rhs_chunks[c][:, half]
        then identity-accumulated extras."""
        nc = self.nc
        n_c, n_extra = len(wtiles), len(extra) if extra else 0
        for mt in range(len(psum)):
            for half in range(0, TOK, 512):
                for c in range(n_c):
                    w = wtiles[c][mt]
                    nc.tensor.matmul(
                        psum[mt][:, half:half + 512], w,
                        rhs_chunks[c][:w.shape[0], half:half + 512],
                        start=(c == 0), stop=(c == n_c - 1 and n_extra == 0))
                if extra:
                    for e, (elhsT, erhs) in enumerate(extra):
                        nc.tensor.matmul(
                            psum[mt][:, half:half + 512], elhsT,
                            erhs[mt][:, half:half + 512],
                            start=False, stop=(e == n_extra - 1))

    def _transpose(self, src_fm, tag, evac_engine="vector"):
        """8 PE transposes of (128,128) b-blocks + one evacuation."""
        nc, g = self.nc, self.g
        ps = self.pools["ps_small"].tile([128, B, 128], F32, tag="ps_sm",
                                         name="ps_tr")
        for b in range(B):
            nc.tensor.matmul(ps[:, b, :], src_fm[:, b, :], g["I"],
                             start=True, stop=True)
        out = self.pools["sb"].tile([128, B, 128], BF16, tag=tag, name=tag)
        if evac_engine == "vector":
            nc.vector.tensor_copy(out, ps)
        else:
            nc.scalar.copy(out, ps)
        return out

    def _cheb(self, nm_sb, tag, evac_engine="vector"):
        """zz (128, 2, B, 128): [S z | 2S^2 z] feature-major, from node-major
        stationaries nm_sb."""
        nc, g = self.nc, self.g
        zz = self.pools["sb"].tile([128, 2, B, 128], BF16, tag=tag, name=tag)
        for halfb in range(2):
            ps = self.pools["ps_small"].tile([128, B // 2, 256], F32,
                                             tag="ps_sm", name="ps_ch")
            for i in range(B // 2):
                b = halfb * (B // 2) + i
                nc.tensor.matmul(ps[:, i, :], nm_sb[:, b, :], g["RC"],
                                 start=True, stop=True)
            bsl = slice(halfb * (B // 2), (halfb + 1) * (B // 2))
            view = ps.rearrange("p b (k n) -> p k b n", k=2)
            if evac_engine == "vector":
                nc.vector.tensor_copy(zz[:, :, bsl, :], view)
            else:
                nc.scalar.copy(zz[:, :, bsl, :], view)
        return zz

    def _gru_step(self, layer, h, hnm, xg_sb, xc_sb):
        """One DCGRU step given the state's node-major transpose and the
        x-part contributions (feature-major, bf16).  Updates h in place."""
        nc, g = self.nc, self.g
        sb, ps_out = self.pools["sb"], self.pools["ps_out"]
        Wgh = g["Wg1h"] if layer else g["Wg0h"]
        Wch = g["Wc1h"] if layer else g["Wc0h"]
        lt = f"l{layer}"

        zz = self._cheb(hnm, lt + "zz")
        hv = h.rearrange("p b n -> p (b n)")
        rhs = [hv, zz[:, 0].rearrange("p b n -> p (b n)"),
               zz[:, 1].rearrange("p b n -> p (b n)")]

        ps_g = [ps_out.tile([128, TOK], F32, tag="ps_out", name="ps_g")
                for _ in range(2)]
        self._w_contract(ps_g, Wgh, rhs,
                         extra=[(g["I"], [xg_sb[:, 0], xg_sb[:, 1]])])
        r = sb.tile([128, TOK], BF16, tag=lt + "r", name=lt + "r")
        u = sb.tile([128, TOK], BF16, tag=lt + "u", name=lt + "u")
        nc.scalar.activation(r, ps_g[0], SIG, bias=g["bg"][layer][:, 0:1])
        nc.scalar.activation(u, ps_g[1], SIG, bias=g["bg"][layer][:, 1:2])

        rh = sb.tile([128, B, N], BF16, tag=lt + "rh", name=lt + "rh")
        nc.vector.tensor_mul(rh.rearrange("p b n -> p (b n)"), r, hv)

        rhnm = self._transpose(rh, lt + "rhnm")
        zzc = self._cheb(rhnm, lt + "zzc")
        ps_c = [ps_out.tile([128, TOK], F32, tag="ps_out", name="ps_c")]
        self._w_contract(ps_c, Wch,
                         [rh.rearrange("p b n -> p (b n)"),
                          zzc[:, 0].rearrange("p b n -> p (b n)"),
                          zzc[:, 1].rearrange("p b n -> p (b n)")],
                         extra=[(g["I"], [xc_sb])])
        c = sb.tile([128, TOK], BF16, tag=lt + "c", name=lt + "c")
        nc.scalar.activation(c, ps_c[0], TANH, bias=g["bc"][layer][:, 0:1])

        # h' = c + u*(h-c)
        d = sb.tile([128, TOK], BF16, tag=lt + "d", name=lt + "d")
        nc.vector.scalar_tensor_tensor(out=d, in0=hv, scalar=0.0, in1=c,
                                       op0=mybir.AluOpType.bypass,
                                       op1=mybir.AluOpType.subtract)
        e = sb.tile([128, TOK], BF16, tag=lt + "e", name=lt + "e")
        nc.vector.tensor_mul(e, u, d)
        nc.vector.tensor_add(hv, e, c)

    def _l1bulk(self, yfm, ynm):
        """Layer-1 x-part from layer-0's state (SBUF only).  Returns
        (xg1_sb (128,2,TOK), xc1_sb (128,TOK)) bf16."""
        nc, g = self.nc, self.g
        sb, ps_out = self.pools["sb"], self.pools["ps_out"]
        zz = self._cheb(ynm, "b1zz", evac_engine="scalar")
        rhs = [yfm.rearrange("p b n -> p (b n)"),
               zz[:, 0].rearrange("p b n -> p (b n)"),
               zz[:, 1].rearrange("p b n -> p (b n)")]
        ps_g = [ps_out.tile([128, TOK], F32, tag="ps_out", name="b1ps_g")
                for _ in range(2)]
        self._w_contract(ps_g, g["Wg1x"], rhs)
        xg1 = sb.tile([128, 2, TOK], BF16, tag="xg1", name="xg1")
        nc.scalar.copy(xg1[:, 0], ps_g[0])
        nc.vector.tensor_copy(xg1[:, 1], ps_g[1])
        ps_c = [ps_out.tile([128, TOK], F32, tag="ps_out", name="b1ps_c")]
        self._w_contract(ps_c, g["Wc1x"], rhs)
        xc1 = sb.tile([128, TOK], BF16, tag="xc1", name="xc1")
        nc.vector.tensor_copy(xc1, ps_c[0])
        return xg1, xc1

    def _bulk_l0_step(self, t, XG0, XC0):
        """Layer-0 x-part for one timestep -> XG0[t]/XC0[t] in DRAM."""
        nc, g = self.nc, self.g
        sb, ps_small, ps_out = (self.pools["sb"], self.pools["ps_small"],
                                self.pools["ps_out"])
        xnm = sb.tile([N, B, D], BF16, tag="xnm", name="xnm")
        nc.sync.dma_start(out=xnm, in_=self.x[t].rearrange("b n d -> n b d"))
        xfm = sb.tile([D, B, N], BF16, tag="xfm", name="xfm")
        nc.sync.dma_start(out=xfm, in_=self.xT[t])

        y1 = sb.tile([64, B, N], BF16, tag="zx1", name="zx1")
        y2 = sb.tile([64, B, N], BF16, tag="zx2", name="zx2")
        for halfb in range(2):
            ps_ch = ps_small.tile([64, B // 2, 256], F32, tag="ps_sm",
                                  name="b0ps_ch")
            for i in range(B // 2):
                b = halfb * (B // 2) + i
                nc.tensor.matmul(ps_ch[:, i, :], xnm[:, b, :], g["RC"],
                                 start=True, stop=True)
            bsl = slice(halfb * (B // 2), (halfb + 1) * (B // 2))
            nc.vector.tensor_copy(y1[:, bsl, :], ps_ch[:, :, 0:128])
            nc.vector.tensor_copy(y2[:, bsl, :], ps_ch[:, :, 128:256])

        rhs = [xfm.rearrange("p b n -> p (b n)"),
               y1.rearrange("p b n -> p (b n)"),
               y2.rearrange("p b n -> p (b n)")]
        ps_g = [ps_out.tile([128, TOK], F32, tag="ps_out", name="b0ps_g")
                for _ in range(2)]
        self._w_contract(ps_g, g["Wg0x"], rhs)
        xg_sb = sb.tile([128, 2, TOK], BF16, tag="bxg", name="bxg")
        nc.scalar.copy(xg_sb[:, 0], ps_g[0])
        nc.scalar.copy(xg_sb[:, 1], ps_g[1])
        nc.sync.dma_start(out=XG0[t], in_=xg_sb)

        ps_c = [ps_out.tile([128, TOK], F32, tag="ps_out", name="b0ps_c")]
        self._w_contract(ps_c, g["Wc0x"], rhs)
        xc_sb = sb.tile([128, TOK], BF16, tag="bxc", name="bxc")
        nc.vector.tensor_copy(xc_sb, ps_c[0])
        nc.sync.dma_start(out=XC0[t], in_=xc_sb)

    # ---------------- whole program ----------------

    def _emit(self, tc):
        nc = self.nc
        import contextlib
        with contextlib.ExitStack() as ctx:
            consts = ctx.enter_context(tc.tile_pool(name="consts", bufs=1))
            dramp = ctx.enter_context(tc.tile_pool(name="dram", bufs=1, space="DRAM"))
            sb = ctx.enter_context(tc.tile_pool(name="sb", bufs=2))
            sb_state = ctx.enter_context(tc.tile_pool(name="sb_state", bufs=1))
            ps_small = ctx.enter_context(
                tc.tile_pool(name="ps_small", bufs=2, space="PSUM"))
            ps_out = ctx.enter_context(
                tc.tile_pool(name="ps_out", bufs=2, space="PSUM"))
            self.pools = dict(sb=sb, ps_small=ps_small, ps_out=ps_out)
            self.g = self._load_const_tiles(consts)
            g = self.g

            XG0 = dramp.tile([T, 128, 2, TOK], BF16, tag="XG0", name="XG0")
            XC0 = dramp.tile([T, 128, TOK], BF16, tag="XC0", name="XC0")

            hfm = [sb_state.tile([H, B, N], BF16, tag=f"hfm{l}", name=f"hfm{l}")
                   for l in range(2)]
            for l in range(2):
                nc.sync.dma_start(out=hfm[l], in_=self.h0fm[l])

            # phase A: layer-0 x-part bulk
            for t in range(T):
                self._bulk_l0_step(t, XG0, XC0)

            # phase B: pipelined rounds
            out_nbh = self.outputs.rearrange("t b (n hh) -> t n b hh", n=N)
            for t in range(T + 1):
                # 1) transpose of h0 = y0_{t-1}
                hnm0 = self._transpose(hfm[0], "hnm0")
                if t == T:
                    # final h0 -> out_hidden[0]
                    nc.gpsimd.dma_start(
                        out=self.out_hidden[0].rearrange("b (n hh) -> n b hh", n=N),
                        in_=hnm0)
                # 2) layer-1 x-part from y0_{t-1}
                if t >= 1:
                    xg1, xc1 = self._l1bulk(hfm[0], hnm0)
                # 3) layer-0 step t
                if t <= T - 1:
                    xg_sb = sb.tile([128, 2, TOK], BF16, tag="xg0", name="xg0")
                    nc.sync.dma_start(out=xg_sb, in_=XG0[t])
                    xc_sb = sb.tile([128, TOK], BF16, tag="xc0", name="xc0")
                    nc.sync.dma_start(out=xc_sb, in_=XC0[t])
                    self._gru_step(0, hfm[0], hnm0, xg_sb, xc_sb)
                # 4) layer-1 step t-1
                if t >= 1:
                    hnm1 = self._transpose(hfm[1], "hnm1", evac_engine="scalar")
                    if t >= 2:
                        # hnm1 holds y1_{t-2}
                        nc.gpsimd.dma_start(out=out_nbh[t - 2], in_=hnm1)
                    self._gru_step(1, hfm[1], hnm1, xg1, xc1)

            # tail: y1_{T-1} = final h1 -> outputs[T-1] + out_hidden[1]
            hnm1 = self._transpose(hfm[1], "hnm1")
            nc.gpsimd.dma_start(out=out_nbh[T - 1], in_=hnm1)
            nc.gpsimd.dma_start(
                out=self.out_hidden[1].rearrange("b (n hh) -> n b hh", n=N),
                in_=hnm1)


_CACHE = {}


def _get_nc():
    if "nc" not in _CACHE:
        _CACHE["nc"] = _Builder().build()
    return _CACHE["nc"]


def kernel(inputs, initial_hidden_state, supports, Wg0, bg0, Wc0, bc0,
           Wg1, bg1, Wc1, bc1):
    nc = _get_nc()
    w = _prep_weights(supports, Wg0, Wc0, Wg1, Wc1)

    x = np.asarray(inputs, dtype=np.float32)                 # (T, 64, N, D)
    h0 = np.asarray(initial_hidden_state, dtype=np.float32).reshape(2, BFULL, N, H)

    in_maps = []
    for cix in range(NCORES):
        bs = slice(cix * B, (cix + 1) * B)
        xs = np.ascontiguousarray(x[:, bs]).astype(nbf)      # (T, B, N, D)
        m = dict(w)
        m["x"] = xs
        m["xT"] = np.ascontiguousarray(xs.transpose(0, 3, 1, 2))
        m["h0fm"] = np.ascontiguousarray(
            h0[:, bs].transpose(0, 3, 1, 2)).astype(nbf)
        m["bg0"] = np.asarray(bg0, np.float32)
        m["bc0"] = np.asarray(bc0, np.float32)
        m["bg1"] = np.asarray(bg1, np.float32)
        m["bc1"] = np.asarray(bc1, np.float32)
        in_maps.append(m)

    _CACHE["last_in_maps"] = in_maps
    res = run_bass_kernel_spmd(nc, in_maps, core_ids=list(range(NCORES)))
    outputs = np.concatenate([r["outputs"] for r in res.results], axis=1).astype(np.float32)
    output_hidden = np.concatenate([r["out_hidden"] for r in res.results], axis=1).astype(np.float32)
    return output_hidden, outputs
